# revision 18
# baseline (speedup 1.0000x reference)
"""Multi-head causal attention (B=2, T=2048, D=1024, H=16, Dh=64) on 8 TRN2
NeuronCores via Bass/Tile. fp16 main path + fp8-DoubleRow attention-weight
matmul.

Sharding: core c -> (batch b = c//4, head group hg = c%4, heads 4*hg..4*hg+3).
Each core computes its 4 heads' attention for its batch plus the partial
output projection over those heads' dims; the host sums the 4 partials per
batch and adds the output bias.

Precision plan (max rel err budget 2e-2; every fp8 stage on the main data
path costs ~2e-2, so):
  - x, Wq/Wk/Wv/Wo, Q^T/K^T staging, ctx: fp16 (PE cost 1 cycle/row, same as
    bf16, 8x less noise).
  - attention weights p: fp8e4m3 (~1e-2 total). That enables the A.V matmul
    in fp8 DoubleRow mode (0.5 cycles/row, 2 k-blocks per matmul = 4x
    cheaper than fp16) with V as an fp8 residual PAIR (v_hi + v_lo, two
    accumulating chains) so V itself contributes ~0.4% like fp16.
  - causal mask: PE matmuls (identity x mask-pattern, -30000 additive).
  - softmax: exp(s/8 - 2) on ACT -> fp8, plus a tunable fraction of score
    tiles on DVE via a bit-exact uint8 fast-exp (b = round(1.4427*s+33.375),
    verified round-to-nearest+saturate on HW); fast-exp only for q-chunks
    c >= 1 so a fully-saturated-to-zero row (few-key rows) cannot occur.
  - denominator: ones column in v_hi -> ctx PSUM row 64; DVE reciprocal ->
    PE ones-broadcast -> fp16 ctx copy -> DVE multiply.
  - DoubleRow quirks measured on this stack: stationary must be [K, 2, 128]
    (M=128 exactly); with K<128 partitions only k-tile 0 is processed.
"""

import numpy as np

D_MODEL = 1024
N_HEADS = 16
D_HEAD = 64
B = 2
T = 2048
N_CORES = 8
HPC = 4            # heads per core
MPC = HPC * D_HEAD # head dims per core = 256
NEG = -30000.0     # additive causal mask in fp16
FEXP_A = 1.44269504   # fast-exp bits = round(raw * A + B), e4m3 bit trick
FEXP_B = 33.375       # 56 + 8*0.0573 - 16/ln2  (folds the -2 exp bias)

# engine assignment knobs (tuned against TimelineSim)
FASTEXP_8THS = 2   # of every 8 eligible score tiles, this many on DVE
QK_COPY_DVE = True
V_COPY_DVE = True
CTXU_COPY_DVE = False  # ACT
O_COPY_DVE = False     # ACT
MASKS_ON_PE = True

PROFILE = False
LAST_RESULTS = None
_CACHE = {}


def _split_waits(nc, mybir, max_waits=1):
    """This walrus build rejects instructions carrying more than `max_waits`
    semaphore waits. Move the excess onto InstNoOp carriers inserted just
    before the instruction on the same engine (same blocking semantics)."""
    for func in nc.m.functions:
        for bb in func.blocks:
            todo = [
                inst for inst in bb.instructions
                if inst.sync_info is not None
                and inst.sync_info.on_wait
                and len(inst.sync_info.on_wait) > max_waits
            ]
            if not todo:
                continue
            carriers = {}
            for inst in todo:
                si = inst.sync_info
                waits = list(si.on_wait)
                si.on_wait = waits[-max_waits:]
                excess = waits[:-max_waits]
                chunks = []
                for i in range(0, len(excess), max_waits):
                    chunk = excess[i: i + max_waits]
                    bi = nc.engines[inst.engine].nop(nofuse=True)
                    nop_inst = bi.ins
                    cur = nc.cur_bb.bb
                    assert cur.instructions[-1] is nop_inst
                    cur.instructions = cur.instructions[:-1]
                    nop_inst.sync_info = mybir.SyncInfo(on_wait=chunk, on_update=[])
                    chunks.append(nop_inst)
                carriers[id(inst)] = chunks
            new_list = []
            for inst in bb.instructions:
                new_list.extend(carriers.get(id(inst), ()))
                new_list.append(inst)
            bb.instructions = new_list


def _build_nc(with_bias=False):
    import concourse.bass as bass
    import concourse.mybir as mybir

    f32 = mybir.dt.float32
    f32r = mybir.dt.float32r
    f16 = mybir.dt.float16
    f8 = mybir.dt.float8e4
    u8 = mybir.dt.uint8
    bf16 = mybir.dt.bfloat16
    DR = mybir.MatmulPerfMode.DoubleRow
    Exp = mybir.ActivationFunctionType.Exp
    Copy = mybir.ActivationFunctionType.Copy
    Identity = mybir.ActivationFunctionType.Identity
    mult = mybir.AluOpType.mult
    add_ = mybir.AluOpType.add
    sub_ = mybir.AluOpType.subtract

    nc = bass.Bass("TRN2", target_bir_lowering=False, debug=False,
                   num_devices=N_CORES)

    x_d = nc.dram_tensor("x16", [128, 8 * T], f16, kind="ExternalInput").ap()
    wq_d = nc.dram_tensor("wq16", [128, 8 * 256], f16, kind="ExternalInput").ap()
    wk_d = nc.dram_tensor("wk16", [128, 8 * 256], f16, kind="ExternalInput").ap()
    wv_d = nc.dram_tensor("wv16", [128, 8 * 260], f16, kind="ExternalInput").ap()
    wo_d = nc.dram_tensor("wo16", [128, 2 * 1024], f16, kind="ExternalInput").ap()
    bv_d = nc.dram_tensor("bvr", [1, 260], f32r, kind="ExternalInput").ap()
    id_d = nc.dram_tensor("id16", [128, 128], f16, kind="ExternalInput").ap()
    mA_d = nc.dram_tensor("mA16", [128, 128], f16, kind="ExternalInput").ap()
    mB_d = nc.dram_tensor("mB16", [128, 256], f16, kind="ExternalInput").ap()
    ones_d = nc.dram_tensor("onesr", [1, 512], f32r, kind="ExternalInput").ap()
    ebias_d = nc.dram_tensor("ebias", [128, 1], f32, kind="ExternalInput").ap()
    z8_d = nc.dram_tensor("z8", [128, 4096], f8, kind="ExternalInput").ap()
    bq_d = nc.dram_tensor("bqc", [128, 2], f32, kind="ExternalInput").ap()
    bk_d = nc.dram_tensor("bkc", [128, 2], f32, kind="ExternalInput").ap()
    out_d = nc.dram_tensor("out", [T, D_MODEL], bf16, kind="ExternalOutput").ap()

    import concourse.tile as tile
    with tile.TileContext(nc) as tc:
        with tc.tile_pool(name="med", bufs=1) as med, \
             tc.tile_pool(name="small", bufs=1) as small, \
             tc.tile_pool(name="xq", bufs=2) as xq, \
             tc.tile_pool(name="p8p", bufs=8) as p8p, \
             tc.tile_pool(name="aux", bufs=4) as aux, \
             tc.tile_pool(name="osb", bufs=6) as osbp, \
             tc.tile_pool(name="sps", bufs=2, space="PSUM") as spool, \
             tc.tile_pool(name="ctxp", bufs=2, space="PSUM") as cpool, \
             tc.tile_pool(name="pr", bufs=2, space="PSUM") as prpool:

            wq_sb = med.tile([128, 2048], f16, tag="wq", name="wq_sb")
            wk_sb = med.tile([128, 2048], f16, tag="wk", name="wk_sb")
            wv_sb = med.tile([128, 2080], f16, tag="wv", name="wv_sb")
            wo_sb = med.tile([128, 2048], f16, tag="wo", name="wo_sb")
            q16 = [med.tile([128, T], f16, tag=f"q{p}", name=f"q16_{p}")
                   for p in range(2)]
            k16 = [med.tile([128, T], f16, tag=f"k{p}", name=f"k16_{p}")
                   for p in range(2)]
            # v8: [g 8][r 2][u 2 (hi/lo)][4 heads x 128-padded unit]
            v8 = med.tile([128, 8 * 2 * 2 * 512], f8, tag="v8", name="v8")
            ctx16 = med.tile([128, 2 * T], f16, tag="ctx16", name="ctx16")
            bvr = small.tile([1, 260], f32r, tag="bvr", name="bvr")
            id16 = small.tile([128, 128], f16, tag="id16", name="id16")
            mA16 = small.tile([128, 128], f16, tag="mA16", name="mA16")
            mB16 = small.tile([128, 256], f16, tag="mB16", name="mB16")
            ones = small.tile([1, 512], f32r, tag="ones", name="ones")
            ebias = small.tile([128, 1], f32, tag="ebias", name="ebias")
            bqc = small.tile([128, 2], f32, tag="bqc", name="bqc")
            bkc = small.tile([128, 2], f32, tag="bkc", name="bkc")

            v8r = v8[:].rearrange("p (g r u m) -> p g r u m", g=8, r=2, u=2)

            def emit_v8_pad_zeros():
                # zero the per-head pad columns [65:128] of every v8 unit
                # once (DMA APs are limited to partition + 2 free dims -> one
                # DMA per (g, r, u) block). Emitted after proj(0) so these 32
                # transfers don't head-of-line-block the startup x/weight
                # DMAs; they only need to land before the first AV matmul.
                zsrc = z8_d[:, 0:252].rearrange("p (h j) -> p h j", h=4)
                for g_ in range(8):
                    for r_ in range(2):
                        for u in range(2):
                            nc.sync.dma_start(
                                v8r[:, g_, r_, u, :]
                                .rearrange("p (h m) -> p h m", h=4)
                                [:, :, 65:128],
                                zsrc,
                            )

            def copy_plain(dve, dst, src, bias_ap=None):
                if dve:
                    if bias_ap is None:
                        nc.vector.tensor_copy(dst, src)
                    else:
                        nc.vector.tensor_scalar(dst, src, bias_ap, None, add_)
                else:
                    if bias_ap is None:
                        nc.scalar.activation(dst, src, Copy)
                    else:
                        nc.scalar.activation(dst, src, Identity, bias=bias_ap)

            def emit_x(nt):
                x16q = xq.tile([128, 8 * 512], f16, tag="x16q", name=f"x16q{nt}")
                nc.sync.dma_start(
                    x16q[:].rearrange("p (dc t) -> p dc t", dc=8),
                    x_d.rearrange("p (dc t) -> p dc t", dc=8)
                    [:, :, nt * 512:(nt + 1) * 512],
                )
                return x16q

            def proj_chains(nt, x16q):
                """The quarter's 8 projection chains as closures, for
                interleaving between attention heads."""
                xv = x16q[:].rearrange("p (dc t) -> p dc t", dc=8)
                chains = []

                def qk_chain(w_sb, dsts, bcol, pt):
                    wv_ = w_sb[:].rearrange("p (dc m) -> p dc m", dc=8)
                    pp = prpool.tile([128, 512], f32, tag="pr", bufs=2,
                                     name="qk_ps")
                    for dc in range(8):
                        nc.tensor.matmul(
                            pp[:],
                            wv_[:, dc, pt * 128:(pt + 1) * 128],
                            xv[:, dc, :],
                            start=(dc == 0), stop=(dc == 7),
                        )
                    bias_ap = bcol[:, pt:pt + 1] if with_bias else None
                    copy_plain(QK_COPY_DVE,
                               dsts[pt][:, nt * 512:(nt + 1) * 512],
                               pp[:], bias_ap)

                def v_chain(tb):
                    wvv = wv_sb[:].rearrange("p (dc m) -> p dc m", dc=8)
                    tl = tb % 4
                    vps = prpool.tile([128, 512], f32, tag="pr", bufs=2,
                                      name="v_ps")
                    for dc in range(8):
                        nc.tensor.matmul(
                            vps[:, 0:260],
                            xv[:, dc, tl * 128:(tl + 1) * 128],
                            wvv[:, dc, :],
                            start=(dc == 0), stop=False,
                        )
                    nc.tensor.matmul(
                        vps[:, 0:260], ones[0:1, 0:128], bvr[:],
                        start=False, stop=True,
                    )
                    g_, r_ = tb // 2, tb % 2
                    hi = (v8r[:, g_, r_, 0, :]
                          .rearrange("p (h m) -> p h m", h=4)[:, :, 0:65])
                    lo = (v8r[:, g_, r_, 1, :]
                          .rearrange("p (h m) -> p h m", h=4)[:, :, 0:65])
                    src = vps[:, 0:260].rearrange("p (h m) -> p h m", h=4)
                    copy_plain(V_COPY_DVE, hi, src)
                    nc.vector.scalar_tensor_tensor(
                        lo, src, 1.0, hi, mult, sub_)

                for (w_sb, dsts, bcol) in ((wq_sb, q16, bqc),
                                           (wk_sb, k16, bkc)):
                    for pt in range(2):
                        chains.append(
                            lambda w=w_sb, d=dsts, b=bcol, p=pt:
                            qk_chain(w, d, b, p))
                for tb in range(4 * nt, 4 * nt + 4):
                    chains.append(lambda t=tb: v_chain(t))
                return chains

            def emit_proj(nt, x16q):
                for ch in proj_chains(nt, x16q):
                    ch()

            fexp_ctr = [0]

            def emit_attn(qb, fillers=()):
                fillers = list(fillers)
                # PE executes its queue in-order, so the A.V matmuls (which
                # wait on the exp of their tile) are emitted one tile LATE and
                # each head's normalization (which waits on the DVE recip) one
                # head late -- the PE always has independent score matmuls in
                # front of any instruction that blocks on ACT/DVE results.
                pending_av = [None]
                pending_norm = [None]

                def flush(slot):
                    if slot[0] is not None:
                        slot[0]()
                        slot[0] = None

                def emit_norm(h, ctx_t):
                    pt, hl = h // 2, h % 2
                    psl = slice(hl * 64, hl * 64 + 64)
                    rec = aux.tile([1, 512], f32r, tag="rec", name="rec")
                    with nc.allow_low_precision(reason="softmax denom recip"):
                        nc.vector.reciprocal(out=rec[:], in_=ctx_t[64:65, :])
                    rbt = prpool.tile([128, 512], f32, tag="pr", bufs=2,
                                      name="rb_ps")
                    nc.tensor.matmul(rbt[:], ones[0:1, 0:128], rec[:],
                                     start=True, stop=True)
                    ctxu = aux.tile([128, 512], f16, tag="ctxu", name="ctxu")
                    if CTXU_COPY_DVE:
                        nc.vector.tensor_copy(ctxu[psl, :], ctx_t[0:64, :])
                    else:
                        nc.scalar.activation(ctxu[psl, :], ctx_t[0:64, :], Copy)
                    nc.vector.tensor_tensor(
                        ctx16[psl, pt * T + qb * 512: pt * T + (qb + 1) * 512],
                        ctxu[psl, :], rbt[psl, :], mult)

                for h in range(HPC):
                    pt, hl = h // 2, h % 2
                    psl = slice(hl * 64, hl * 64 + 64)
                    ctx_t = cpool.tile([128, 512], f32, tag="ctx", bufs=2,
                                       name="ctx_ps")
                    for ci, c in enumerate((2 * qb, 2 * qb + 1)):
                        qoff = 256 * c
                        csl = slice(ci * 256, (ci + 1) * 256)
                        # one (scores, exp, AV) stage per k-GROUP of 2 blocks:
                        # 1-bank PSUM tiles give 4 pipeline slots so the PE
                        # runs several groups ahead of the exps
                        for g in range(c + 1):
                            sps = spool.tile([128, 512], f32, tag="sps",
                                             bufs=4, name="s_ps")
                            for j in range(2):
                                kb = 2 * g + j
                                diag = kb >= 2 * c
                                nc.tensor.matmul(
                                    sps[:, j * 256:(j + 1) * 256],
                                    k16[pt][psl, kb * 128:(kb + 1) * 128],
                                    q16[pt][psl, qoff:qoff + 256],
                                    start=True, stop=not diag,
                                )
                                if kb == 2 * c:
                                    nc.tensor.matmul(
                                        sps[:, j * 256:j * 256 + 128],
                                        id16[:], mA16[:],
                                        start=False, stop=True,
                                        skip_group_check=True,
                                    )
                                elif kb == 2 * c + 1:
                                    nc.tensor.matmul(
                                        sps[:, j * 256:(j + 1) * 256],
                                        id16[:], mB16[:],
                                        start=False, stop=True,
                                        skip_group_check=True,
                                    )
                            p8t = p8p.tile([128, 512], f8, tag="p8",
                                           name="p8t")
                            use_dve = (c >= 1
                                       and (fexp_ctr[0] % 8) < FASTEXP_8THS)
                            fexp_ctr[0] += 1
                            if use_dve:
                                nc.vector.tensor_scalar(
                                    p8t[:].bitcast(u8), sps[:],
                                    FEXP_A, FEXP_B, mult, add_)
                            else:
                                nc.scalar.activation(
                                    p8t[:], sps[:], Exp,
                                    scale=0.125, bias=ebias[:])
                            flush(pending_av)

                            def av(h=h, c=c, g=g, p8t=p8t, ctx_t=ctx_t,
                                   csl=csl):
                                rhs_p = p8t[:].rearrange(
                                    "p (r n) -> p r n", r=2)
                                for u in range(2):
                                    nc.tensor.matmul(
                                        ctx_t[:, csl],
                                        v8r[:, g, :, u,
                                            h * 128:(h + 1) * 128],
                                        rhs_p,
                                        start=(g == 0 and u == 0),
                                        stop=(g == c and u == 1),
                                        perf_mode=DR,
                                    )
                            pending_av[0] = av
                        if fillers:
                            fillers.pop(0)()
                    flush(pending_norm)
                    flush(pending_av)
                    pending_norm[0] = (lambda h=h, ctx_t=ctx_t:
                                       emit_norm(h, ctx_t))
                flush(pending_norm)
                for f in fillers:
                    f()

            def out_units(tb0, tb1):
                ctxv = ctx16[:].rearrange("p (r t) -> p r t", r=2)
                wov = wo_sb[:].rearrange("p (r n) -> p r n", r=2)
                units = []
                for tb in range(tb0, tb1):
                    for on in range(2):
                        units.append(lambda t=tb, o=on: out_unit(t, o))
                return units

            def out_unit(tb, on):
                ctxv = ctx16[:].rearrange("p (r t) -> p r t", r=2)
                wov = wo_sb[:].rearrange("p (r n) -> p r n", r=2)
                if True:
                    if True:
                        ops = prpool.tile([128, 512], f32, tag="pr", bufs=2,
                                          name="o_ps")
                        for r in range(2):
                            nc.tensor.matmul(
                                ops[:],
                                ctxv[:, r, tb * 128:(tb + 1) * 128],
                                wov[:, r, on * 512:(on + 1) * 512],
                                start=(r == 0), stop=(r == 1),
                            )
                        osb = osbp.tile([128, 512], bf16, tag="osb",
                                        name="o_sb")
                        if (tb * 2 + on) % 2 == (1 if O_COPY_DVE else 0):
                            nc.vector.tensor_copy(osb[:], ops[:])
                        else:
                            nc.scalar.activation(osb[:], ops[:], Copy)
                        nc.sync.dma_start(
                            out_d[tb * 128:(tb + 1) * 128,
                                  on * 512:(on + 1) * 512],
                            osb[:])

            def emit_out(tb0, tb1):
                for ch in out_units(tb0, tb1):
                    ch()

            # ---- software pipeline ----
            # attn(qb) depends only on proj(<=qb) (keys/values up to the
            # causal frontier live in quarters <= qb), so each attn phase is
            # emitted right after its quarter's projections; O-projection
            # chunks are spread between phases to keep the ACT/DVE queues
            # from head-of-line-blocking the exps that pace the PE.
            x0 = emit_x(0)
            nc.sync.dma_start(wq_sb[:], wq_d)
            nc.sync.dma_start(wk_sb[:], wk_d)
            nc.sync.dma_start(wv_sb[:], wv_d)
            nc.sync.dma_start(bvr[:], bv_d)
            nc.sync.dma_start(ones[:], ones_d)
            nc.sync.dma_start(ebias[:], ebias_d)
            if with_bias:
                nc.sync.dma_start(bqc[:], bq_d)
                nc.sync.dma_start(bkc[:], bk_d)
            emit_proj(0, x0)
            nc.sync.dma_start(id16[:], id_d)
            nc.sync.dma_start(mA16[:], mA_d)
            nc.sync.dma_start(mB16[:], mB_d)
            nc.sync.dma_start(wo_sb[:], wo_d)
            emit_v8_pad_zeros()
            x1 = emit_x(1)
            emit_attn(0, proj_chains(1, x1))
            x2 = emit_x(2)
            emit_attn(1, proj_chains(2, x2))
            x3 = emit_x(3)
            emit_attn(2, proj_chains(3, x3) + out_units(0, 4))
            emit_attn(3, out_units(4, 8) + out_units(8, 12))
            emit_out(12, 16)

    _split_waits(nc, mybir)
    # This walrus build cannot encode EVENT_SEMAPHORE_RANGE_CLEAR.
    for bb in nc.m.functions[0].blocks:
        bb.instructions = [
            inst for inst in bb.instructions
            if getattr(inst, "op_name", None) != "EVENT_SEMAPHORE_RANGE_CLEAR"
        ]
    return nc


def _get_nc(with_bias=False):
    key = ("nc", with_bias)
    if key not in _CACHE:
        _CACHE[key] = _build_nc(with_bias)
    return _CACHE[key]


def _chunk_pack(mat, ncols):
    """[1024, ncols] -> [128, 8, ncols]: contraction chunk-major."""
    return np.ascontiguousarray(
        mat.reshape(8, 128, ncols).transpose(1, 0, 2))


def _masks():
    ki = np.arange(128)[:, None]
    jA = np.arange(128)[None, :]
    jB = np.arange(256)[None, :]
    mA = np.where(jA >= ki, 0.0, NEG).astype(np.float32)
    mB = np.where(jB >= ki + 128, 0.0, NEG).astype(np.float32)
    return mA, mB


def kernel(x, Wq, bq, Wk, bk, Wv, bv, Wo, bo):
    global LAST_RESULTS
    from concourse.bass_utils import run_bass_kernel_spmd
    import concourse.mybir as mybir

    f16np = np.float16
    f8np = mybir.dt.np(mybir.dt.float8e4)

    x = np.asarray(x, np.float32)
    Wq = np.asarray(Wq, np.float32)
    Wk = np.asarray(Wk, np.float32)
    Wv = np.asarray(Wv, np.float32)
    Wo = np.asarray(Wo, np.float32)
    bq = np.asarray(bq, np.float32)
    bk = np.asarray(bk, np.float32)
    bv = np.asarray(bv, np.float32)
    bo = np.asarray(bo, np.float32)

    wqT, wkT, wvT, woT = Wq.T, Wk.T, Wv.T, Wo.T
    mA, mB = _masks()
    id16 = np.eye(128, dtype=f16np)
    onesr = np.ones((1, 512), np.float32)
    ebias = np.full((128, 1), -2.0, np.float32)
    z8 = np.zeros((128, 4096), np.float32).astype(f8np)

    x16s = []
    for b in range(B):
        xt = np.ascontiguousarray(x[b].T)
        x16s.append(_chunk_pack(xt, T).reshape(128, 8 * T).astype(f16np))

    per_hg = {}
    for hg in range(HPC):
        sl = slice(hg * MPC, (hg + 1) * MPC)
        wq16 = _chunk_pack(wqT[:, sl], 256).reshape(128, 2048).astype(f16np)
        wk16 = _chunk_pack(wkT[:, sl], 256).reshape(128, 2048).astype(f16np)
        wva = np.zeros((D_MODEL, 260), np.float32)
        for j in range(HPC):
            wva[:, j * 65: j * 65 + 64] = wvT[:, hg * MPC + j * 64:
                                              hg * MPC + (j + 1) * 64]
        wv16 = _chunk_pack(wva, 260).reshape(128, 2080).astype(f16np)
        bvrow = np.zeros((1, 260), np.float32)
        for j in range(HPC):
            bvrow[0, j * 65: j * 65 + 64] = bv[hg * MPC + j * 64:
                                               hg * MPC + (j + 1) * 64]
            bvrow[0, j * 65 + 64] = 1.0
        # wo16 [128, 2, 1024]: ctx dim at (p, r) = 64*(2r + p//64) + p%64
        rows = woT[sl, :]
        wo16 = np.empty((128, 2, 1024), np.float32)
        for r in range(2):
            for hf in range(2):
                wo16[hf * 64:(hf + 1) * 64, r, :] = \
                    rows[(2 * r + hf) * 64:(2 * r + hf + 1) * 64, :]
        wo16 = wo16.reshape(128, 2048).astype(f16np)
        bqcol = np.ascontiguousarray(bq[sl].reshape(2, 128).T)
        bkcol = np.ascontiguousarray(bk[sl].reshape(2, 128).T)
        per_hg[hg] = (wq16, wk16, wv16, bvrow, wo16, bqcol, bkcol)

    in_maps = []
    for c in range(N_CORES):
        b, hg = c // 4, c % 4
        wq16, wk16, wv16, bvrow, wo16, bqcol, bkcol = per_hg[hg]
        in_maps.append({
            "x16": x16s[b], "wq16": wq16, "wk16": wk16, "wv16": wv16,
            "wo16": wo16, "bvr": bvrow, "id16": id16,
            "mA16": mA.astype(f16np), "mB16": mB.astype(f16np),
            "onesr": onesr, "ebias": ebias, "z8": z8,
            "bqc": bqcol, "bkc": bkcol,
        })

    with_bias = bool(np.any(bq != 0.0) or np.any(bk != 0.0))
    nc = _get_nc(with_bias)
    res = run_bass_kernel_spmd(
        nc, in_maps, list(range(N_CORES)), trace=PROFILE,
    )
    LAST_RESULTS = res

    out = np.zeros((B, T, D_MODEL), np.float32)
    for c in range(N_CORES):
        out[c // 4] += np.asarray(res.results[c]["out"], dtype=np.float32)
    out += bo
    return out


# revision 19
# speedup vs baseline: 1.0152x; 1.0152x over previous
"""Multi-head causal attention (B=2, T=2048, D=1024, H=16, Dh=64) on 8 TRN2
NeuronCores via Bass/Tile. fp16 main path + fp8-DoubleRow attention-weight
matmul.

Sharding: core c -> (batch b = c//4, head group hg = c%4, heads 4*hg..4*hg+3).
Each core computes its 4 heads' attention for its batch plus the partial
output projection over those heads' dims; the host sums the 4 partials per
batch and adds the output bias.

Precision plan (max rel err budget 2e-2; every fp8 stage on the main data
path costs ~2e-2, so):
  - x, Wq/Wk/Wv/Wo, Q^T/K^T staging, ctx: fp16 (PE cost 1 cycle/row, same as
    bf16, 8x less noise).
  - attention weights p: fp8e4m3 (~1e-2 total). That enables the A.V matmul
    in fp8 DoubleRow mode (0.5 cycles/row, 2 k-blocks per matmul = 4x
    cheaper than fp16) with V as an fp8 residual PAIR (v_hi + v_lo, two
    accumulating chains) so V itself contributes ~0.4% like fp16.
  - causal mask: PE matmuls (identity x mask-pattern, -30000 additive).
  - softmax: exp(s/8 - 2) on ACT -> fp8, plus a tunable fraction of score
    tiles on DVE via a bit-exact uint8 fast-exp (b = round(1.4427*s+33.375),
    verified round-to-nearest+saturate on HW); fast-exp only for q-chunks
    c >= 1 so a fully-saturated-to-zero row (few-key rows) cannot occur.
  - denominator: ones column in v_hi -> ctx PSUM row 64; DVE reciprocal ->
    PE ones-broadcast -> fp16 ctx copy -> DVE multiply.
  - DoubleRow quirks measured on this stack: stationary must be [K, 2, 128]
    (M=128 exactly); with K<128 partitions only k-tile 0 is processed.
"""

import numpy as np

D_MODEL = 1024
N_HEADS = 16
D_HEAD = 64
B = 2
T = 2048
N_CORES = 8
HPC = 4            # heads per core
MPC = HPC * D_HEAD # head dims per core = 256
NEG = -30000.0     # additive causal mask in fp16
FEXP_A = 1.44269504   # fast-exp bits = round(raw * A + B), e4m3 bit trick
FEXP_B = 33.375       # 56 + 8*0.0573 - 16/ln2  (folds the -2 exp bias)

# engine assignment knobs (tuned against TimelineSim)
FASTEXP_8THS = 3   # of every 8 eligible score tiles, this many on DVE
QK_COPY_DVE = True
V_COPY_DVE = True
CTXU_COPY_DVE = False  # ACT
O_COPY_DVE = False     # ACT
MASKS_ON_PE = True

PROFILE = False
LAST_RESULTS = None
_CACHE = {}


def _split_waits(nc, mybir, max_waits=1):
    """This walrus build rejects instructions carrying more than `max_waits`
    semaphore waits. Move the excess onto InstNoOp carriers inserted just
    before the instruction on the same engine (same blocking semantics)."""
    for func in nc.m.functions:
        for bb in func.blocks:
            todo = [
                inst for inst in bb.instructions
                if inst.sync_info is not None
                and inst.sync_info.on_wait
                and len(inst.sync_info.on_wait) > max_waits
            ]
            if not todo:
                continue
            carriers = {}
            for inst in todo:
                si = inst.sync_info
                waits = list(si.on_wait)
                si.on_wait = waits[-max_waits:]
                excess = waits[:-max_waits]
                chunks = []
                for i in range(0, len(excess), max_waits):
                    chunk = excess[i: i + max_waits]
                    bi = nc.engines[inst.engine].nop(nofuse=True)
                    nop_inst = bi.ins
                    cur = nc.cur_bb.bb
                    assert cur.instructions[-1] is nop_inst
                    cur.instructions = cur.instructions[:-1]
                    nop_inst.sync_info = mybir.SyncInfo(on_wait=chunk, on_update=[])
                    chunks.append(nop_inst)
                carriers[id(inst)] = chunks
            new_list = []
            for inst in bb.instructions:
                new_list.extend(carriers.get(id(inst), ()))
                new_list.append(inst)
            bb.instructions = new_list


def _build_nc(with_bias=False):
    import concourse.bass as bass
    import concourse.mybir as mybir

    f32 = mybir.dt.float32
    f32r = mybir.dt.float32r
    f16 = mybir.dt.float16
    f8 = mybir.dt.float8e4
    u8 = mybir.dt.uint8
    bf16 = mybir.dt.bfloat16
    DR = mybir.MatmulPerfMode.DoubleRow
    Exp = mybir.ActivationFunctionType.Exp
    Copy = mybir.ActivationFunctionType.Copy
    Identity = mybir.ActivationFunctionType.Identity
    mult = mybir.AluOpType.mult
    add_ = mybir.AluOpType.add
    sub_ = mybir.AluOpType.subtract

    nc = bass.Bass("TRN2", target_bir_lowering=False, debug=False,
                   num_devices=N_CORES)

    x_d = nc.dram_tensor("x16", [128, 8 * T], f16, kind="ExternalInput").ap()
    wq_d = nc.dram_tensor("wq16", [128, 8 * 256], f16, kind="ExternalInput").ap()
    wk_d = nc.dram_tensor("wk16", [128, 8 * 256], f16, kind="ExternalInput").ap()
    wv_d = nc.dram_tensor("wv16", [128, 8 * 260], f16, kind="ExternalInput").ap()
    wo_d = nc.dram_tensor("wo16", [128, 2 * 1024], f16, kind="ExternalInput").ap()
    bv_d = nc.dram_tensor("bvr", [1, 260], f32r, kind="ExternalInput").ap()
    id_d = nc.dram_tensor("id16", [128, 128], f16, kind="ExternalInput").ap()
    mA_d = nc.dram_tensor("mA16", [128, 128], f16, kind="ExternalInput").ap()
    mB_d = nc.dram_tensor("mB16", [128, 256], f16, kind="ExternalInput").ap()
    ones_d = nc.dram_tensor("onesr", [1, 512], f32r, kind="ExternalInput").ap()
    ebias_d = nc.dram_tensor("ebias", [128, 1], f32, kind="ExternalInput").ap()
    z8_d = nc.dram_tensor("z8", [128, 4096], f8, kind="ExternalInput").ap()
    bq_d = nc.dram_tensor("bqc", [128, 2], f32, kind="ExternalInput").ap()
    bk_d = nc.dram_tensor("bkc", [128, 2], f32, kind="ExternalInput").ap()
    out_d = nc.dram_tensor("out", [T, D_MODEL], bf16, kind="ExternalOutput").ap()

    import concourse.tile as tile
    with tile.TileContext(nc) as tc:
        with tc.tile_pool(name="med", bufs=1) as med, \
             tc.tile_pool(name="small", bufs=1) as small, \
             tc.tile_pool(name="xq", bufs=2) as xq, \
             tc.tile_pool(name="p8p", bufs=8) as p8p, \
             tc.tile_pool(name="aux", bufs=4) as aux, \
             tc.tile_pool(name="osb", bufs=6) as osbp, \
             tc.tile_pool(name="sps", bufs=2, space="PSUM") as spool, \
             tc.tile_pool(name="ctxp", bufs=2, space="PSUM") as cpool, \
             tc.tile_pool(name="pr", bufs=2, space="PSUM") as prpool:

            wq_sb = med.tile([128, 2048], f16, tag="wq", name="wq_sb")
            wk_sb = med.tile([128, 2048], f16, tag="wk", name="wk_sb")
            wv_sb = med.tile([128, 2080], f16, tag="wv", name="wv_sb")
            wo_sb = med.tile([128, 2048], f16, tag="wo", name="wo_sb")
            q16 = [med.tile([128, T], f16, tag=f"q{p}", name=f"q16_{p}")
                   for p in range(2)]
            k16 = [med.tile([128, T], f16, tag=f"k{p}", name=f"k16_{p}")
                   for p in range(2)]
            # v8: [g 8][r 2][u 2 (hi/lo)][4 heads x 128-padded unit]
            v8 = med.tile([128, 8 * 2 * 2 * 512], f8, tag="v8", name="v8")
            ctx16 = med.tile([128, 2 * T], f16, tag="ctx16", name="ctx16")
            bvr = small.tile([1, 260], f32r, tag="bvr", name="bvr")
            id16 = small.tile([128, 128], f16, tag="id16", name="id16")
            mA16 = small.tile([128, 128], f16, tag="mA16", name="mA16")
            mB16 = small.tile([128, 256], f16, tag="mB16", name="mB16")
            ones = small.tile([1, 512], f32r, tag="ones", name="ones")
            ebias = small.tile([128, 1], f32, tag="ebias", name="ebias")
            bqc = small.tile([128, 2], f32, tag="bqc", name="bqc")
            bkc = small.tile([128, 2], f32, tag="bkc", name="bkc")

            v8r = v8[:].rearrange("p (g r u m) -> p g r u m", g=8, r=2, u=2)

            def emit_v8_pad_zeros():
                # zero the per-head pad columns [65:128] of every v8 unit
                # once (DMA APs are limited to partition + 2 free dims -> one
                # DMA per (g, r, u) block). Emitted after proj(0) so these 32
                # transfers don't head-of-line-block the startup x/weight
                # DMAs; they only need to land before the first AV matmul.
                zsrc = z8_d[:, 0:252].rearrange("p (h j) -> p h j", h=4)
                for g_ in range(8):
                    for r_ in range(2):
                        for u in range(2):
                            nc.sync.dma_start(
                                v8r[:, g_, r_, u, :]
                                .rearrange("p (h m) -> p h m", h=4)
                                [:, :, 65:128],
                                zsrc,
                            )

            def copy_plain(dve, dst, src, bias_ap=None):
                if dve:
                    if bias_ap is None:
                        nc.vector.tensor_copy(dst, src)
                    else:
                        nc.vector.tensor_scalar(dst, src, bias_ap, None, add_)
                else:
                    if bias_ap is None:
                        nc.scalar.activation(dst, src, Copy)
                    else:
                        nc.scalar.activation(dst, src, Identity, bias=bias_ap)

            def emit_x(nt):
                x16q = xq.tile([128, 8 * 512], f16, tag="x16q", name=f"x16q{nt}")
                nc.sync.dma_start(
                    x16q[:].rearrange("p (dc t) -> p dc t", dc=8),
                    x_d.rearrange("p (dc t) -> p dc t", dc=8)
                    [:, :, nt * 512:(nt + 1) * 512],
                )
                return x16q

            def proj_chains(nt, x16q):
                """The quarter's 8 projection chains as closures, for
                interleaving between attention heads."""
                xv = x16q[:].rearrange("p (dc t) -> p dc t", dc=8)
                chains = []

                def qk_chain(w_sb, dsts, bcol, pt):
                    wv_ = w_sb[:].rearrange("p (dc m) -> p dc m", dc=8)
                    pp = prpool.tile([128, 512], f32, tag="pr", bufs=2,
                                     name="qk_ps")
                    for dc in range(8):
                        nc.tensor.matmul(
                            pp[:],
                            wv_[:, dc, pt * 128:(pt + 1) * 128],
                            xv[:, dc, :],
                            start=(dc == 0), stop=(dc == 7),
                        )
                    bias_ap = bcol[:, pt:pt + 1] if with_bias else None
                    copy_plain(QK_COPY_DVE,
                               dsts[pt][:, nt * 512:(nt + 1) * 512],
                               pp[:], bias_ap)

                def v_chain(tb):
                    wvv = wv_sb[:].rearrange("p (dc m) -> p dc m", dc=8)
                    tl = tb % 4
                    vps = prpool.tile([128, 512], f32, tag="pr", bufs=2,
                                      name="v_ps")
                    for dc in range(8):
                        nc.tensor.matmul(
                            vps[:, 0:260],
                            xv[:, dc, tl * 128:(tl + 1) * 128],
                            wvv[:, dc, :],
                            start=(dc == 0), stop=False,
                        )
                    nc.tensor.matmul(
                        vps[:, 0:260], ones[0:1, 0:128], bvr[:],
                        start=False, stop=True,
                    )
                    g_, r_ = tb // 2, tb % 2
                    hi = (v8r[:, g_, r_, 0, :]
                          .rearrange("p (h m) -> p h m", h=4)[:, :, 0:65])
                    lo = (v8r[:, g_, r_, 1, :]
                          .rearrange("p (h m) -> p h m", h=4)[:, :, 0:65])
                    src = vps[:, 0:260].rearrange("p (h m) -> p h m", h=4)
                    copy_plain(V_COPY_DVE, hi, src)
                    nc.vector.scalar_tensor_tensor(
                        lo, src, 1.0, hi, mult, sub_)

                for (w_sb, dsts, bcol) in ((wq_sb, q16, bqc),
                                           (wk_sb, k16, bkc)):
                    for pt in range(2):
                        chains.append(
                            lambda w=w_sb, d=dsts, b=bcol, p=pt:
                            qk_chain(w, d, b, p))
                for tb in range(4 * nt, 4 * nt + 4):
                    chains.append(lambda t=tb: v_chain(t))
                return chains

            def emit_proj(nt, x16q):
                for ch in proj_chains(nt, x16q):
                    ch()

            fexp_ctr = [0]

            def emit_attn(qb, fillers=()):
                fillers = list(fillers)
                # PE executes its queue in-order, so the A.V matmuls (which
                # wait on the exp of their tile) are emitted one tile LATE and
                # each head's normalization (which waits on the DVE recip) one
                # head late -- the PE always has independent score matmuls in
                # front of any instruction that blocks on ACT/DVE results.
                pending_av = [None]
                pending_norm = [None]

                def flush(slot):
                    if slot[0] is not None:
                        slot[0]()
                        slot[0] = None

                def emit_norm(h, ctx_t):
                    pt, hl = h // 2, h % 2
                    psl = slice(hl * 64, hl * 64 + 64)
                    rec = aux.tile([1, 512], f32r, tag="rec", name="rec")
                    with nc.allow_low_precision(reason="softmax denom recip"):
                        nc.vector.reciprocal(out=rec[:], in_=ctx_t[64:65, :])
                    rbt = prpool.tile([128, 512], f32, tag="pr", bufs=2,
                                      name="rb_ps")
                    nc.tensor.matmul(rbt[:], ones[0:1, 0:128], rec[:],
                                     start=True, stop=True)
                    ctxu = aux.tile([128, 512], f16, tag="ctxu", name="ctxu")
                    if CTXU_COPY_DVE:
                        nc.vector.tensor_copy(ctxu[psl, :], ctx_t[0:64, :])
                    else:
                        nc.scalar.activation(ctxu[psl, :], ctx_t[0:64, :], Copy)
                    nc.vector.tensor_tensor(
                        ctx16[psl, pt * T + qb * 512: pt * T + (qb + 1) * 512],
                        ctxu[psl, :], rbt[psl, :], mult)

                for h in range(HPC):
                    pt, hl = h // 2, h % 2
                    psl = slice(hl * 64, hl * 64 + 64)
                    ctx_t = cpool.tile([128, 512], f32, tag="ctx", bufs=2,
                                       name="ctx_ps")
                    for ci, c in enumerate((2 * qb, 2 * qb + 1)):
                        qoff = 256 * c
                        csl = slice(ci * 256, (ci + 1) * 256)
                        # one (scores, exp, AV) stage per k-GROUP of 2 blocks:
                        # 1-bank PSUM tiles give 4 pipeline slots so the PE
                        # runs several groups ahead of the exps
                        for g in range(c + 1):
                            sps = spool.tile([128, 512], f32, tag="sps",
                                             bufs=4, name="s_ps")
                            for j in range(2):
                                kb = 2 * g + j
                                diag = kb >= 2 * c
                                nc.tensor.matmul(
                                    sps[:, j * 256:(j + 1) * 256],
                                    k16[pt][psl, kb * 128:(kb + 1) * 128],
                                    q16[pt][psl, qoff:qoff + 256],
                                    start=True, stop=not diag,
                                )
                                if kb == 2 * c:
                                    nc.tensor.matmul(
                                        sps[:, j * 256:j * 256 + 128],
                                        id16[:], mA16[:],
                                        start=False, stop=True,
                                        skip_group_check=True,
                                    )
                                elif kb == 2 * c + 1:
                                    nc.tensor.matmul(
                                        sps[:, j * 256:(j + 1) * 256],
                                        id16[:], mB16[:],
                                        start=False, stop=True,
                                        skip_group_check=True,
                                    )
                            p8t = p8p.tile([128, 512], f8, tag="p8",
                                           name="p8t")
                            use_dve = (c >= 1
                                       and (fexp_ctr[0] % 8) < FASTEXP_8THS)
                            fexp_ctr[0] += 1
                            if use_dve:
                                nc.vector.tensor_scalar(
                                    p8t[:].bitcast(u8), sps[:],
                                    FEXP_A, FEXP_B, mult, add_)
                            else:
                                nc.scalar.activation(
                                    p8t[:], sps[:], Exp,
                                    scale=0.125, bias=ebias[:])
                            flush(pending_av)

                            def av(h=h, c=c, g=g, p8t=p8t, ctx_t=ctx_t,
                                   csl=csl):
                                rhs_p = p8t[:].rearrange(
                                    "p (r n) -> p r n", r=2)
                                for u in range(2):
                                    nc.tensor.matmul(
                                        ctx_t[:, csl],
                                        v8r[:, g, :, u,
                                            h * 128:(h + 1) * 128],
                                        rhs_p,
                                        start=(g == 0 and u == 0),
                                        stop=(g == c and u == 1),
                                        perf_mode=DR,
                                    )
                            pending_av[0] = av
                        if fillers:
                            fillers.pop(0)()
                    flush(pending_norm)
                    flush(pending_av)
                    pending_norm[0] = (lambda h=h, ctx_t=ctx_t:
                                       emit_norm(h, ctx_t))
                flush(pending_norm)
                for f in fillers:
                    f()

            def out_units(tb0, tb1):
                ctxv = ctx16[:].rearrange("p (r t) -> p r t", r=2)
                wov = wo_sb[:].rearrange("p (r n) -> p r n", r=2)
                units = []
                for tb in range(tb0, tb1):
                    for on in range(2):
                        units.append(lambda t=tb, o=on: out_unit(t, o))
                return units

            def out_unit(tb, on):
                ctxv = ctx16[:].rearrange("p (r t) -> p r t", r=2)
                wov = wo_sb[:].rearrange("p (r n) -> p r n", r=2)
                if True:
                    if True:
                        ops = prpool.tile([128, 512], f32, tag="pr", bufs=2,
                                          name="o_ps")
                        for r in range(2):
                            nc.tensor.matmul(
                                ops[:],
                                ctxv[:, r, tb * 128:(tb + 1) * 128],
                                wov[:, r, on * 512:(on + 1) * 512],
                                start=(r == 0), stop=(r == 1),
                            )
                        osb = osbp.tile([128, 512], bf16, tag="osb",
                                        name="o_sb")
                        if (tb * 2 + on) % 2 == (1 if O_COPY_DVE else 0):
                            nc.vector.tensor_copy(osb[:], ops[:])
                        else:
                            nc.scalar.activation(osb[:], ops[:], Copy)
                        nc.sync.dma_start(
                            out_d[tb * 128:(tb + 1) * 128,
                                  on * 512:(on + 1) * 512],
                            osb[:])

            def emit_out(tb0, tb1):
                for ch in out_units(tb0, tb1):
                    ch()

            # ---- software pipeline ----
            # attn(qb) depends only on proj(<=qb) (keys/values up to the
            # causal frontier live in quarters <= qb), so each attn phase is
            # emitted right after its quarter's projections; O-projection
            # chunks are spread between phases to keep the ACT/DVE queues
            # from head-of-line-blocking the exps that pace the PE.
            x0 = emit_x(0)
            nc.sync.dma_start(wq_sb[:], wq_d)
            nc.sync.dma_start(wk_sb[:], wk_d)
            nc.sync.dma_start(wv_sb[:], wv_d)
            nc.sync.dma_start(bvr[:], bv_d)
            nc.sync.dma_start(ones[:], ones_d)
            nc.sync.dma_start(ebias[:], ebias_d)
            if with_bias:
                nc.sync.dma_start(bqc[:], bq_d)
                nc.sync.dma_start(bkc[:], bk_d)
            emit_proj(0, x0)
            nc.sync.dma_start(id16[:], id_d)
            nc.sync.dma_start(mA16[:], mA_d)
            nc.sync.dma_start(mB16[:], mB_d)
            nc.sync.dma_start(wo_sb[:], wo_d)
            emit_v8_pad_zeros()
            x1 = emit_x(1)
            emit_attn(0, proj_chains(1, x1))
            x2 = emit_x(2)
            emit_attn(1, proj_chains(2, x2))
            x3 = emit_x(3)
            emit_attn(2, proj_chains(3, x3) + out_units(0, 4))
            emit_attn(3, out_units(4, 8) + out_units(8, 12))
            emit_out(12, 16)

    _split_waits(nc, mybir)
    # This walrus build cannot encode EVENT_SEMAPHORE_RANGE_CLEAR.
    for bb in nc.m.functions[0].blocks:
        bb.instructions = [
            inst for inst in bb.instructions
            if getattr(inst, "op_name", None) != "EVENT_SEMAPHORE_RANGE_CLEAR"
        ]
    return nc


def _get_nc(with_bias=False):
    key = ("nc", with_bias)
    if key not in _CACHE:
        _CACHE[key] = _build_nc(with_bias)
    return _CACHE[key]


def _chunk_pack(mat, ncols):
    """[1024, ncols] -> [128, 8, ncols]: contraction chunk-major."""
    return np.ascontiguousarray(
        mat.reshape(8, 128, ncols).transpose(1, 0, 2))


def _masks():
    ki = np.arange(128)[:, None]
    jA = np.arange(128)[None, :]
    jB = np.arange(256)[None, :]
    mA = np.where(jA >= ki, 0.0, NEG).astype(np.float32)
    mB = np.where(jB >= ki + 128, 0.0, NEG).astype(np.float32)
    return mA, mB


def kernel(x, Wq, bq, Wk, bk, Wv, bv, Wo, bo):
    global LAST_RESULTS
    from concourse.bass_utils import run_bass_kernel_spmd
    import concourse.mybir as mybir

    f16np = np.float16
    f8np = mybir.dt.np(mybir.dt.float8e4)

    x = np.asarray(x, np.float32)
    Wq = np.asarray(Wq, np.float32)
    Wk = np.asarray(Wk, np.float32)
    Wv = np.asarray(Wv, np.float32)
    Wo = np.asarray(Wo, np.float32)
    bq = np.asarray(bq, np.float32)
    bk = np.asarray(bk, np.float32)
    bv = np.asarray(bv, np.float32)
    bo = np.asarray(bo, np.float32)

    wqT, wkT, wvT, woT = Wq.T, Wk.T, Wv.T, Wo.T
    mA, mB = _masks()
    id16 = np.eye(128, dtype=f16np)
    onesr = np.ones((1, 512), np.float32)
    ebias = np.full((128, 1), -2.0, np.float32)
    z8 = np.zeros((128, 4096), np.float32).astype(f8np)

    x16s = []
    for b in range(B):
        xt = np.ascontiguousarray(x[b].T)
        x16s.append(_chunk_pack(xt, T).reshape(128, 8 * T).astype(f16np))

    per_hg = {}
    for hg in range(HPC):
        sl = slice(hg * MPC, (hg + 1) * MPC)
        wq16 = _chunk_pack(wqT[:, sl], 256).reshape(128, 2048).astype(f16np)
        wk16 = _chunk_pack(wkT[:, sl], 256).reshape(128, 2048).astype(f16np)
        wva = np.zeros((D_MODEL, 260), np.float32)
        for j in range(HPC):
            wva[:, j * 65: j * 65 + 64] = wvT[:, hg * MPC + j * 64:
                                              hg * MPC + (j + 1) * 64]
        wv16 = _chunk_pack(wva, 260).reshape(128, 2080).astype(f16np)
        bvrow = np.zeros((1, 260), np.float32)
        for j in range(HPC):
            bvrow[0, j * 65: j * 65 + 64] = bv[hg * MPC + j * 64:
                                               hg * MPC + (j + 1) * 64]
            bvrow[0, j * 65 + 64] = 1.0
        # wo16 [128, 2, 1024]: ctx dim at (p, r) = 64*(2r + p//64) + p%64
        rows = woT[sl, :]
        wo16 = np.empty((128, 2, 1024), np.float32)
        for r in range(2):
            for hf in range(2):
                wo16[hf * 64:(hf + 1) * 64, r, :] = \
                    rows[(2 * r + hf) * 64:(2 * r + hf + 1) * 64, :]
        wo16 = wo16.reshape(128, 2048).astype(f16np)
        bqcol = np.ascontiguousarray(bq[sl].reshape(2, 128).T)
        bkcol = np.ascontiguousarray(bk[sl].reshape(2, 128).T)
        per_hg[hg] = (wq16, wk16, wv16, bvrow, wo16, bqcol, bkcol)

    in_maps = []
    for c in range(N_CORES):
        b, hg = c // 4, c % 4
        wq16, wk16, wv16, bvrow, wo16, bqcol, bkcol = per_hg[hg]
        in_maps.append({
            "x16": x16s[b], "wq16": wq16, "wk16": wk16, "wv16": wv16,
            "wo16": wo16, "bvr": bvrow, "id16": id16,
            "mA16": mA.astype(f16np), "mB16": mB.astype(f16np),
            "onesr": onesr, "ebias": ebias, "z8": z8,
            "bqc": bqcol, "bkc": bkcol,
        })

    with_bias = bool(np.any(bq != 0.0) or np.any(bk != 0.0))
    nc = _get_nc(with_bias)
    res = run_bass_kernel_spmd(
        nc, in_maps, list(range(N_CORES)), trace=PROFILE,
    )
    LAST_RESULTS = res

    out = np.zeros((B, T, D_MODEL), np.float32)
    for c in range(N_CORES):
        out[c // 4] += np.asarray(res.results[c]["out"], dtype=np.float32)
    out += bo
    return out
